# revision 22
# baseline (speedup 1.0000x reference)
"""Chamfer distance (squared L2) on 8 Trainium2 NeuronCores.

Problem: xyz1 [16, 4096, 3], xyz2 [16, 4096, 3] fp32.
  d[b,n,m] = ||xyz1[b,n] - xyz2[b,m]||^2
  out = (mean_{b,n} min_m d, mean_{b,m} min_n d)

Sharding: data-parallel over batch, 2 batches per core. Each core computes
its batches' sum-of-row-mins and sum-of-col-mins; host combines means.

Per-core algorithm (per batch):
  - Augmented K=7 float32r matmul produces distance tiles in PSUM:
      lhsT rows: [x~, y~, z~, s1h, s1l, 1, 1]  (x~ = f32r-rounded coords)
      rhs  rows: [-2x~', -2y~', -2z~', 1, 1, s2h, s2l]
    where sh = f32r(||rounded point||^2), sl = f32r(s - sh). Consistent
    rounding + hi/lo norm rows => the matmul yields the squared distance
    between the rounded points to ~1e-8, so no additive per-pair noise
    biases the min selection (single-rounded norms cost 8% error here).
  - dist1 (min over m): per [128, 1024] PSUM group, DVE tensor_reduce
    computes the per-row min into an accumulator column.
  - dist2 (min over n): running elementwise min across the 32 n-chunks,
    kept in per-m-quarter fp32 SBUF buffers, updated by DVE tensor_tensor
    min directly from PSUM. (tensor_tensor_reduce would fuse these two
    passes but faults the device on this toolchain; GPSIMD tensor_tensor
    supports only add/mult, so everything min-shaped runs on DVE.)
  - Finals: PE transposes of the running-min buffers + DVE free-axis min
    reduce the partition direction; sums via a ones-vector matmul.
"""

import numpy as np
from contextlib import ExitStack

import concourse.bacc as bacc
import concourse.tile as tile
import concourse.mybir as mybir
from concourse import masks
from concourse import bass_utils

F32 = mybir.dt.float32
F32R = mybir.dt.float32r
MIN = mybir.AluOpType.min
MULT = mybir.AluOpType.mult
ADD = mybir.AluOpType.add
AX_X = mybir.AxisListType.X
SQUARE = mybir.ActivationFunctionType.Square

P = 128          # partitions / n-chunk size
FREE = 512       # matmul free dim = one PSUM bank of fp32
GW = 1024        # group width (2 PSUM banks) consumed per DVE op

# Problem shape (hardcoded per contest contract)
B_FULL, N_PTS, M_PTS, D = 16, 4096, 4096, 3
N_CORES = 8
BPC = B_FULL // N_CORES  # batches per core

def _build(bpc, n, m, reps=1):
    """Build the SPMD program for `bpc` batches of [3, n] x [3, m] points."""
    nt_cnt = n // P           # n-chunks
    ng = m // GW              # m-quarters (groups per n-chunk)
    trc = GW // P             # transposes per R buffer

    nc = bacc.Bacc("TRN2", target_bir_lowering=False, debug=False)
    x1d = nc.dram_tensor("x1", [bpc, D, n], F32, kind="ExternalInput")
    x2d = nc.dram_tensor("x2", [bpc, D, m], F32, kind="ExternalInput")
    onesd = nc.dram_tensor("ones", [2, max(n, m)], F32, kind="ExternalInput")
    outd = nc.dram_tensor("out", [2, bpc], F32, kind="ExternalOutput")

    with tile.TileContext(nc) as tc, ExitStack() as ctx:
        consts = ctx.enter_context(tc.tile_pool(name="consts", bufs=1))
        apool = ctx.enter_context(tc.tile_pool(name="aug", bufs=2))
        scratch = ctx.enter_context(tc.tile_pool(name="scratch", bufs=1))
        rpool = ctx.enter_context(tc.tile_pool(name="R", bufs=2 * ng))
        accp = ctx.enter_context(tc.tile_pool(name="acc", bufs=2))
        ps_main = ctx.enter_context(tc.tile_pool(name="psm", bufs=3, space="PSUM"))
        ps_aux = ctx.enter_context(tc.tile_pool(name="psa", bufs=2, space="PSUM"))
        ps_norm = ps_aux
        ps_tr = ps_aux

        ones_d = consts.tile([D, 1], F32)
        nc.gpsimd.memset(ones_d[:], 1.0)
        ones_p = consts.tile([P, 1], F32)
        nc.gpsimd.memset(ones_p[:], 1.0)
        ident = consts.tile([P, P], F32)
        masks.make_identity(nc, ident[:])
        outsb = consts.tile([2, bpc], F32)

        def prep_side(xd, b, cols, neg2, on_dve=False):
            """DMA one side, build its [7, cols] float32r augmented matrix.

            Compute-engine SBUF writes must start at partition 0/32/64/96, so
            rows 3-6 cannot be written directly. Instead all pieces are staged
            fp32 at base partition 0, assembled into afp with SBUF->SBUF DMAs
            (no partition restriction), and a single ACT copy rounds the whole
            [7, cols] matrix to float32r (which also satisfies the verifier's
            "f32r operands must come from a rounding op" rule).
            """
            t0 = scratch.tile([D, cols], F32, tag="t0")
            nc.sync.dma_start(t0[:], xd[b])
            cr = scratch.tile([D, cols], F32R, tag="cr")
            sq = scratch.tile([D, cols], F32, tag="t0", name="sq")
            if neg2:
                # f32r(-2x) = -2 * f32r(x) exactly (power-of-two scale)
                if on_dve:
                    nc.vector.tensor_scalar_mul(cr[:], t0[:], -2.0)
                    nc.vector.tensor_tensor(sq[:], cr[:].bitcast(F32),
                                            cr[:].bitcast(F32), MULT)
                    nc.vector.tensor_scalar_mul(sq[:], sq[:], 0.25)
                else:
                    nc.scalar.mul(cr[:], t0[:], -2.0)
                    # x~^2 from the scaled rounded rows: Square((-2x~)*-0.5)
                    nc.scalar.activation(sq[:], cr[:].bitcast(F32), SQUARE,
                                         scale=-0.5)
                r_norm, r_one = 5, 3
            else:
                if on_dve:
                    nc.vector.tensor_copy(cr[:], t0[:])
                    nc.vector.tensor_tensor(sq[:], cr[:].bitcast(F32),
                                            cr[:].bitcast(F32), MULT)
                else:
                    nc.scalar.copy(cr[:], t0[:])
                    nc.scalar.activation(sq[:], cr[:].bitcast(F32), SQUARE)
                r_norm, r_one = 3, 5
            # norm row: sum the D squared rows with a tiny fp32 ones-matmul
            nrow = scratch.tile([1, cols], F32, tag="nrow")
            for j in range(cols // FREE):
                pn = ps_norm.tile([1, FREE], F32, tag="aux", name="pn")
                nc.tensor.matmul(pn[:], ones_d[:],
                                 sq[:, j * FREE:(j + 1) * FREE],
                                 start=True, stop=True)
                nc.scalar.copy(nrow[:, j * FREE:(j + 1) * FREE], pn[:])
            # hi/lo split: sh = f32r(nrow); nrow <- nrow - sh (lo part, fp32;
            # the final rounding copy below turns it into f32r(s - sh))
            sh = scratch.tile([1, cols], F32R, tag="sh")
            nc.scalar.copy(sh[:], nrow[:])
            nc.gpsimd.tensor_sub(nrow[:], nrow[:], sh[:].bitcast(F32))
            # assemble fp32 staging matrix via DMA, then round to f32r
            afp = scratch.tile([7, cols], F32, tag="afp")
            nc.sync.dma_start(afp[0:D, :], cr[:].bitcast(F32))
            nc.sync.dma_start(afp[r_norm:r_norm + 1, :], sh[:].bitcast(F32))
            nc.sync.dma_start(afp[r_norm + 1:r_norm + 2, :], nrow[:])
            nc.sync.dma_start(afp[r_one:r_one + 2, :], onesd[:, 0:cols])
            A = apool.tile([7, cols], F32R, tag="A2" if neg2 else "A1")
            nc.scalar.copy(A[:], afp[:])
            return A

        def one_rep():
            accs = []
            rbufs = []
            for b in range(bpc):
                # per-batch prep emission: keeps the in-order PE queue free of
                # later batches' norm-matmuls (which wait on their ACT squares)
                A1 = prep_side(x1d, b, n, neg2=False, on_dve=(b == 0))
                A2 = prep_side(x2d, b, m, neg2=True)
                acc1 = accp.tile([P, nt_cnt * ng], F32, tag="acc1",
                                 name=f"acc1_{b}")
                rbuf = [None] * ng  # fp32 running-min buffer per m-quarter

                for nt in range(nt_cnt):
                    for q in range(ng):
                        pm = ps_main.tile([P, GW], F32, tag="pm")
                        for j in range(GW // FREE):
                            mb = q * (GW // FREE) + j
                            nc.tensor.matmul(
                                pm[:, j * FREE:(j + 1) * FREE],
                                A1[:, nt * P:(nt + 1) * P],
                                A2[:, mb * FREE:(mb + 1) * FREE],
                                start=True, stop=True)
                        col = nt * ng + q
                        nc.vector.tensor_reduce(acc1[:, col:col + 1], pm[:],
                                                axis=AX_X, op=MIN)
                        if rbuf[q] is None:
                            rbuf[q] = rpool.tile([P, GW], F32, tag="R",
                                                 name=f"r{b}_{q}")
                            nc.vector.tensor_copy(rbuf[q][:], pm[:])
                        else:
                            nc.vector.tensor_tensor(rbuf[q][:], pm[:],
                                                    rbuf[q][:], MIN)
                accs.append(acc1)
                rbufs.append(rbuf)

            # ---- deferred finals (after all main loops) ----
            for b in range(bpc):
                acc1 = accs[b]
                rbuf = rbufs[b]
                acc2 = accp.tile([P, m // P], F32, tag="acc2", name=f"acc2_{b}")
                tgrp = 4  # transposes batched into one PSUM bank per reduce
                for q in range(ng):
                    rfin = rbuf[q]
                    for t0_ in range(0, trc, tgrp):
                        gsz = min(tgrp, trc - t0_)
                        pt = ps_tr.tile([P, tgrp * P], F32, tag="aux", name="pt")
                        for k in range(gsz):
                            t = t0_ + k
                            nc.tensor.transpose(pt[:, k * P:(k + 1) * P],
                                                rfin[:, t * P:(t + 1) * P],
                                                ident[:])
                        c2 = q * trc + t0_
                        nc.vector.tensor_reduce(
                            acc2[:, c2:c2 + gsz],
                            pt[:, 0:gsz * P].rearrange("p (g c) -> p g c", c=P),
                            axis=AX_X, op=MIN)

                # dist1: min over the ng quarter-columns for each nt, then sum
                d1 = accp.tile([P, nt_cnt], F32, tag="d1", name=f"d1_{b}")
                if ng == 1:
                    nc.vector.tensor_copy(d1[:], acc1[:])
                else:
                    nc.vector.tensor_tensor(d1[:], acc1[:, 0::ng], acc1[:, 1::ng],
                                            MIN)
                    for q in range(2, ng):
                        nc.vector.tensor_tensor(d1[:], d1[:], acc1[:, q::ng], MIN)
                ssum = accp.tile([P, 2], F32, tag="ssum", name=f"ssum_{b}")
                nc.vector.tensor_reduce(ssum[:, 0:1], d1[:], axis=AX_X, op=ADD)
                nc.vector.tensor_reduce(ssum[:, 1:2], acc2[:], axis=AX_X, op=ADD)
                po = ps_norm.tile([2, 1], F32, tag="aux", name="po")
                nc.tensor.matmul(po[:], ssum[:], ones_p[:], start=True, stop=True)
                nc.scalar.copy(outsb[:, b:b + 1], po[:])


        for _rep in range(reps):
            one_rep()

        nc.sync.dma_start(outd[:], outsb[:])

    nc.compile()
    return nc


_NC_CACHE = {}


def _get_nc():
    key = (BPC, N_PTS, M_PTS)
    if key not in _NC_CACHE:
        _NC_CACHE[key] = _build(*key)
    return _NC_CACHE[key]


def run(xyz1, xyz2, trace=False):
    """Run on 8 cores; returns ((mean1, mean2), exec_time_ns_or_None)."""
    x1 = np.ascontiguousarray(
        np.asarray(xyz1, dtype=np.float32).transpose(0, 2, 1))  # [B, 3, N]
    x2 = np.ascontiguousarray(
        np.asarray(xyz2, dtype=np.float32).transpose(0, 2, 1))  # [B, 3, M]
    assert x1.shape == (B_FULL, D, N_PTS) and x2.shape == (B_FULL, D, M_PTS)

    nc = _get_nc()
    ones_row = np.ones((2, max(N_PTS, M_PTS)), dtype=np.float32)
    in_maps = [
        {"x1": np.ascontiguousarray(x1[c * BPC:(c + 1) * BPC]),
         "x2": np.ascontiguousarray(x2[c * BPC:(c + 1) * BPC]),
         "ones": ones_row}
        for c in range(N_CORES)
    ]
    res = bass_utils.run_bass_kernel_spmd(nc, in_maps, list(range(N_CORES)),
                                          trace=trace)
    sum1 = 0.0
    sum2 = 0.0
    for c in range(N_CORES):
        o = np.asarray(res.results[c]["out"], dtype=np.float64)
        sum1 += o[0].sum()
        sum2 += o[1].sum()
    mean1 = np.float32(sum1 / (B_FULL * N_PTS))
    mean2 = np.float32(sum2 / (B_FULL * M_PTS))
    return (mean1, mean2), res.exec_time_ns


def kernel(xyz1, xyz2):
    return run(xyz1, xyz2, trace=False)[0]



# revision 23
# speedup vs baseline: 1.2888x; 1.2888x over previous
"""Chamfer distance (squared L2) on 8 Trainium2 NeuronCores.

Problem: xyz1 [16, 4096, 3], xyz2 [16, 4096, 3] fp32.
  d[b,n,m] = ||xyz1[b,n] - xyz2[b,m]||^2
  out = (mean_{b,n} min_m d, mean_{b,m} min_n d)

Sharding: data-parallel over batch, 2 batches per core. Each core computes
its batches' sum-of-row-mins and sum-of-col-mins; host combines means.

Per-core algorithm (per batch):
  - Augmented K=7 float32r matmul produces distance tiles in PSUM:
      lhsT rows: [x~, y~, z~, s1h, s1l, 1, 1]  (x~ = f32r-rounded coords)
      rhs  rows: [-2x~', -2y~', -2z~', 1, 1, s2h, s2l]
    where sh = f32r(||rounded point||^2), sl = f32r(s - sh). Consistent
    rounding + hi/lo norm rows => the matmul yields the squared distance
    between the rounded points to ~1e-8, so no additive per-pair noise
    biases the min selection (single-rounded norms cost 8% error here).
  - dist1 (min over m): per [128, 1024] PSUM group, DVE tensor_reduce
    computes the per-row min into an accumulator column.
  - dist2 (min over n): running elementwise min across the 32 n-chunks,
    kept in per-m-quarter fp32 SBUF buffers, updated by DVE tensor_tensor
    min directly from PSUM. (tensor_tensor_reduce would fuse these two
    passes but faults the device on this toolchain; GPSIMD tensor_tensor
    supports only add/mult, so everything min-shaped runs on DVE.)
  - Finals: PE transposes of the running-min buffers + DVE free-axis min
    reduce the partition direction; sums via a ones-vector matmul.
"""

import numpy as np
from contextlib import ExitStack

import concourse.bacc as bacc
import concourse.tile as tile
import concourse.mybir as mybir
from concourse import masks
from concourse import bass_utils

F32 = mybir.dt.float32
F32R = mybir.dt.float32r
F16 = mybir.dt.float16
MIN = mybir.AluOpType.min
MULT = mybir.AluOpType.mult
ADD = mybir.AluOpType.add
AX_X = mybir.AxisListType.X
SQUARE = mybir.ActivationFunctionType.Square

P = 128          # partitions / n-chunk size
FREE = 512       # matmul free dim = one PSUM bank of fp32
GW = 1024        # group width (2 PSUM banks) consumed per DVE op

# Problem shape (hardcoded per contest contract)
B_FULL, N_PTS, M_PTS, D = 16, 4096, 4096, 3
N_CORES = 8
BPC = B_FULL // N_CORES  # batches per core

def _build(bpc, n, m, reps=1):
    """Build the SPMD program for `bpc` batches of [3, n] x [3, m] points."""
    nt_cnt = n // P           # n-chunks
    ng = m // GW              # m-quarters (groups per n-chunk)
    trc = GW // P             # transposes per R buffer

    nc = bacc.Bacc("TRN2", target_bir_lowering=False, debug=False)
    x1d = nc.dram_tensor("x1", [bpc, D, n], F32, kind="ExternalInput")
    x2d = nc.dram_tensor("x2", [bpc, D, m], F32, kind="ExternalInput")
    onesd = nc.dram_tensor("ones", [2, max(n, m)], F32, kind="ExternalInput")
    outd = nc.dram_tensor("out", [2, bpc], F32, kind="ExternalOutput")

    with tile.TileContext(nc) as tc, ExitStack() as ctx:
        consts = ctx.enter_context(tc.tile_pool(name="consts", bufs=1))
        apool = ctx.enter_context(tc.tile_pool(name="aug", bufs=2))
        scratch = ctx.enter_context(tc.tile_pool(name="scratch", bufs=1))
        spool = ctx.enter_context(tc.tile_pool(name="S", bufs=3))
        rpool = ctx.enter_context(tc.tile_pool(name="R", bufs=2 * ng))
        accp = ctx.enter_context(tc.tile_pool(name="acc", bufs=2))
        ps_main = ctx.enter_context(tc.tile_pool(name="psm", bufs=3, space="PSUM"))
        ps_aux = ctx.enter_context(tc.tile_pool(name="psa", bufs=2, space="PSUM"))
        ps_norm = ps_aux
        ps_tr = ps_aux

        ones_d = consts.tile([D, 1], F32)
        nc.gpsimd.memset(ones_d[:], 1.0)
        ones_p = consts.tile([P, 1], F32)
        nc.gpsimd.memset(ones_p[:], 1.0)
        ident16 = consts.tile([P, P], F16)
        masks.make_identity(nc, ident16[:])
        outsb = consts.tile([2, bpc], F32)

        def prep_side(xd, b, cols, neg2, on_dve=False):
            """DMA one side, build its [7, cols] float32r augmented matrix.

            Compute-engine SBUF writes must start at partition 0/32/64/96, so
            rows 3-6 cannot be written directly. Instead all pieces are staged
            fp32 at base partition 0, assembled into afp with SBUF->SBUF DMAs
            (no partition restriction), and a single ACT copy rounds the whole
            [7, cols] matrix to float32r (which also satisfies the verifier's
            "f32r operands must come from a rounding op" rule).
            """
            t0 = scratch.tile([D, cols], F32, tag="t0")
            nc.sync.dma_start(t0[:], xd[b])
            cr = scratch.tile([D, cols], F32R, tag="cr")
            sq = scratch.tile([D, cols], F32, tag="t0", name="sq")
            if neg2:
                # f32r(-2x) = -2 * f32r(x) exactly (power-of-two scale)
                if on_dve:
                    nc.vector.tensor_scalar_mul(cr[:], t0[:], -2.0)
                    nc.vector.tensor_tensor(sq[:], cr[:].bitcast(F32),
                                            cr[:].bitcast(F32), MULT)
                    nc.vector.tensor_scalar_mul(sq[:], sq[:], 0.25)
                else:
                    nc.scalar.mul(cr[:], t0[:], -2.0)
                    # x~^2 from the scaled rounded rows: Square((-2x~)*-0.5)
                    nc.scalar.activation(sq[:], cr[:].bitcast(F32), SQUARE,
                                         scale=-0.5)
                r_norm, r_one = 5, 3
            else:
                if on_dve:
                    nc.vector.tensor_copy(cr[:], t0[:])
                    nc.vector.tensor_tensor(sq[:], cr[:].bitcast(F32),
                                            cr[:].bitcast(F32), MULT)
                else:
                    nc.scalar.copy(cr[:], t0[:])
                    nc.scalar.activation(sq[:], cr[:].bitcast(F32), SQUARE)
                r_norm, r_one = 3, 5
            # norm row: sum the D squared rows with a tiny fp32 ones-matmul
            nrow = scratch.tile([1, cols], F32, tag="nrow")
            for j in range(cols // FREE):
                pn = ps_norm.tile([1, FREE], F32, tag="aux", name="pn")
                nc.tensor.matmul(pn[:], ones_d[:],
                                 sq[:, j * FREE:(j + 1) * FREE],
                                 start=True, stop=True)
                nc.scalar.copy(nrow[:, j * FREE:(j + 1) * FREE], pn[:])
            # hi/lo split: sh = f32r(nrow); nrow <- nrow - sh (lo part, fp32;
            # the final rounding copy below turns it into f32r(s - sh))
            sh = scratch.tile([1, cols], F32R, tag="sh")
            nc.scalar.copy(sh[:], nrow[:])
            nc.gpsimd.tensor_sub(nrow[:], nrow[:], sh[:].bitcast(F32))
            # assemble fp32 staging matrix via DMA, then round to f32r
            afp = scratch.tile([7, cols], F32, tag="afp")
            nc.sync.dma_start(afp[0:D, :], cr[:].bitcast(F32))
            nc.sync.dma_start(afp[r_norm:r_norm + 1, :], sh[:].bitcast(F32))
            nc.sync.dma_start(afp[r_norm + 1:r_norm + 2, :], nrow[:])
            nc.sync.dma_start(afp[r_one:r_one + 2, :], onesd[:, 0:cols])
            A = apool.tile([7, cols], F32R, tag="A2" if neg2 else "A1")
            nc.scalar.copy(A[:], afp[:])
            return A

        def one_rep():
            accs = []
            rbufs = []
            for b in range(bpc):
                # per-batch prep emission: keeps the in-order PE queue free of
                # later batches' norm-matmuls (which wait on their ACT squares)
                A1 = prep_side(x1d, b, n, neg2=False, on_dve=(b == 0))
                A2 = prep_side(x2d, b, m, neg2=True)
                acc1 = accp.tile([P, nt_cnt * ng], F32, tag="acc1",
                                 name=f"acc1_{b}")
                rbuf = [None] * ng  # fp32 running-min buffer per m-quarter

                for nt in range(nt_cnt):
                    for q in range(ng):
                        pm = ps_main.tile([P, GW], F32, tag="pm")
                        for j in range(GW // FREE):
                            mb = q * (GW // FREE) + j
                            nc.tensor.matmul(
                                pm[:, j * FREE:(j + 1) * FREE],
                                A1[:, nt * P:(nt + 1) * P],
                                A2[:, mb * FREE:(mb + 1) * FREE],
                                start=True, stop=True)
                        col = nt * ng + q
                        # fused: s = fp16(pm), acc1 col = clean fp32 row-min
                        # (tensor_scalar's accum_out reduces with op1)
                        s = spool.tile([P, GW], F16, tag="S", name="s")
                        nc.vector.tensor_scalar(
                            s[:], pm[:], 0.0, None, op0=ADD, op1=MIN,
                            accum_out=acc1[:, col:col + 1])
                        if rbuf[q] is None:
                            rbuf[q] = rpool.tile([P, GW], F16, tag="R",
                                                 name=f"r{b}_{q}")
                            nc.vector.tensor_copy(rbuf[q][:], s[:])
                        else:
                            nc.vector.tensor_tensor(rbuf[q][:], s[:],
                                                    rbuf[q][:], MIN)
                accs.append(acc1)
                rbufs.append(rbuf)

            # ---- deferred finals (after all main loops) ----
            for b in range(bpc):
                acc1 = accs[b]
                rbuf = rbufs[b]
                acc2 = accp.tile([P, m // P], F32, tag="acc2", name=f"acc2_{b}")
                tgrp = 4  # transposes batched into one PSUM bank per reduce
                for q in range(ng):
                    rfin = rbuf[q]
                    for t0_ in range(0, trc, tgrp):
                        gsz = min(tgrp, trc - t0_)
                        pt = ps_tr.tile([P, tgrp * P], F16, tag="aux", name="pt")
                        for k in range(gsz):
                            t = t0_ + k
                            nc.tensor.transpose(pt[:, k * P:(k + 1) * P],
                                                rfin[:, t * P:(t + 1) * P],
                                                ident16[:])
                        c2 = q * trc + t0_
                        nc.vector.tensor_reduce(
                            acc2[:, c2:c2 + gsz],
                            pt[:, 0:gsz * P].rearrange("p (g c) -> p g c", c=P),
                            axis=AX_X, op=MIN)

                # dist1: min over the ng quarter-columns for each nt, then sum
                d1 = accp.tile([P, nt_cnt], F32, tag="d1", name=f"d1_{b}")
                if ng == 1:
                    nc.vector.tensor_copy(d1[:], acc1[:])
                else:
                    nc.vector.tensor_tensor(d1[:], acc1[:, 0::ng], acc1[:, 1::ng],
                                            MIN)
                    for q in range(2, ng):
                        nc.vector.tensor_tensor(d1[:], d1[:], acc1[:, q::ng], MIN)
                ssum = accp.tile([P, 2], F32, tag="ssum", name=f"ssum_{b}")
                nc.vector.tensor_reduce(ssum[:, 0:1], d1[:], axis=AX_X, op=ADD)
                nc.vector.tensor_reduce(ssum[:, 1:2], acc2[:], axis=AX_X, op=ADD)
                po = ps_norm.tile([2, 1], F32, tag="aux", name="po")
                nc.tensor.matmul(po[:], ssum[:], ones_p[:], start=True, stop=True)
                nc.scalar.copy(outsb[:, b:b + 1], po[:])


        for _rep in range(reps):
            one_rep()

        nc.sync.dma_start(outd[:], outsb[:])

    nc.compile()
    return nc


_NC_CACHE = {}


def _get_nc():
    key = (BPC, N_PTS, M_PTS)
    if key not in _NC_CACHE:
        _NC_CACHE[key] = _build(*key)
    return _NC_CACHE[key]


def run(xyz1, xyz2, trace=False):
    """Run on 8 cores; returns ((mean1, mean2), exec_time_ns_or_None)."""
    x1 = np.ascontiguousarray(
        np.asarray(xyz1, dtype=np.float32).transpose(0, 2, 1))  # [B, 3, N]
    x2 = np.ascontiguousarray(
        np.asarray(xyz2, dtype=np.float32).transpose(0, 2, 1))  # [B, 3, M]
    assert x1.shape == (B_FULL, D, N_PTS) and x2.shape == (B_FULL, D, M_PTS)

    nc = _get_nc()
    ones_row = np.ones((2, max(N_PTS, M_PTS)), dtype=np.float32)
    in_maps = [
        {"x1": np.ascontiguousarray(x1[c * BPC:(c + 1) * BPC]),
         "x2": np.ascontiguousarray(x2[c * BPC:(c + 1) * BPC]),
         "ones": ones_row}
        for c in range(N_CORES)
    ]
    res = bass_utils.run_bass_kernel_spmd(nc, in_maps, list(range(N_CORES)),
                                          trace=trace)
    sum1 = 0.0
    sum2 = 0.0
    for c in range(N_CORES):
        o = np.asarray(res.results[c]["out"], dtype=np.float64)
        sum1 += o[0].sum()
        sum2 += o[1].sum()
    mean1 = np.float32(sum1 / (B_FULL * N_PTS))
    mean2 = np.float32(sum2 / (B_FULL * M_PTS))
    return (mean1, mean2), res.exec_time_ns


def kernel(xyz1, xyz2):
    return run(xyz1, xyz2, trace=False)[0]

